# revision 19
# baseline (speedup 1.0000x reference)
"""Trainium2 kernel for nn_Attention (B=8, S=2048, D=768, H=12, DH=64, R=64).

Sharding: data-parallel over batch -> 1 batch element per NeuronCore (8 cores).
No collectives. LayerNorm affine folded into QKV weights on host.

Architecture (per core): the softmax exp on ScalarE (ACT) is the bottleneck
engine (~1.15us per [128,1024] tile x 384 tiles ~ 440us); everything else is
scheduled as filler around a dense ACT pipeline:

  phase 1 (lead-in): LN -> xT (PE transpose), k/v projections for all S,
    q projection for chunk 0.
  attention (4 chunks of 512 q-positions x 6 head-pairs x 16 key-tiles):
    scores [A|B] -> one PSUM tile [128,1024] (double-buffered, 4 banks)
    exp on ACT -> e [128,1024] bf16 (scale=1/8 folded in)
    ctx accumulation pA/pB [65,512] via ones-column trick (denominator at
    row 64), eagerly evacuated to SBUF (ctxU, bf16) per head-pair.
  normalization: per chunk, batched reciprocal of all 12 denominator rows,
    broadcast across partitions via DRAM round-trip DMA, then per-head
    multiply (A half direct, B half staged + partition-shift DMA).
  filler (interleaved per head-pair into the emission stream so the Tile
    scheduler can fill PE/DVE gaps without starving ACT): q projections for
    chunks 1-3, msa matmuls, adapter, msaT transposes, residual add, out DMA
    of the previous chunk.
"""

import sys

sys.path.insert(0, "/opt/trn_rl_repo")

import numpy as np

import concourse.bass as bass
import concourse.mybir as mybir
import concourse.tile as tile
from concourse import bacc, bass_utils
from concourse.masks import make_identity

F32 = mybir.dt.float32
BF16 = mybir.dt.bfloat16

B, S, D = 8, 2048, 768
H, DH, R = 12, 64, 64
EPS = 1e-6
NCORES = 8

ST = S // 128          # 16 s-tiles
DT = D // 128          # 6 d-tiles
HB = 66                # per-head block width in v (ones | v(64) | ones)
NC_CHUNK = 512         # q-positions per attention chunk
NCHUNK = S // NC_CHUNK # 4


def build_nc() -> bass.Bass:
    nc = bacc.Bacc(None, target_bir_lowering=False, debug=False)

    y_ext = nc.declare_dram_parameter("y", [S, D], F32, isOutput=False)
    qT_kv_ext = nc.declare_dram_parameter("qkv_wT_kv", [D, 2 * D], BF16, isOutput=False)
    qT_q_ext = nc.declare_dram_parameter("qkv_wT_q", [D, D], BF16, isOutput=False)
    qkvb_ext = nc.declare_dram_parameter("qkv_b_eff", [3 * D], F32, isOutput=False)
    msaT_ext = nc.declare_dram_parameter("msa_wT", [D, D], BF16, isOutput=False)
    a1T_ext = nc.declare_dram_parameter("a1_wT", [D, R], BF16, isOutput=False)
    a1b_ext = nc.declare_dram_parameter("a1_b", [R], F32, isOutput=False)
    a2T_ext = nc.declare_dram_parameter("a2_wT_aug", [128, D], BF16, isOutput=False)
    out_ext = nc.declare_dram_parameter("out", [S, D], F32, isOutput=True)

    with tile.TileContext(nc) as tc:
        _build(tc, y_ext, qT_kv_ext, qT_q_ext, qkvb_ext, msaT_ext, a1T_ext,
               a1b_ext, a2T_ext, out_ext)
    nc.compile()
    return nc


def _build(tc, y_ext, qT_kv_ext, qT_q_ext, qkvb_ext, msaT_ext, a1T_ext,
           a1b_ext, a2T_ext, out_ext):
    from contextlib import ExitStack

    nc = tc.nc
    EXP = mybir.ActivationFunctionType.Exp
    RSQRT = mybir.ActivationFunctionType.Rsqrt
    RELU = mybir.ActivationFunctionType.Relu

    with ExitStack() as stack:
        ec = stack.enter_context
        # ---------------- long-lived pools ----------------
        consts = ec(tc.tile_pool(name="consts", bufs=1))
        big = ec(tc.tile_pool(name="big", bufs=1))

        ident = consts.tile([128, 128], BF16)
        make_identity(nc, ident)

        # weights not needed in the first microseconds go on the gpsimd DMA
        # queue so the y tiles aren't stuck behind them on the sync queue
        msaT_sb_w = consts.tile([128, DT, D], BF16)
        nc.gpsimd.dma_start(
            out=msaT_sb_w, in_=msaT_ext.ap().rearrange("(ko p) j -> p ko j", p=128)
        )
        a1T_sb = consts.tile([128, DT, R], BF16)
        nc.gpsimd.dma_start(
            out=a1T_sb, in_=a1T_ext.ap().rearrange("(ko p) j -> p ko j", p=128)
        )
        a2T_sb = consts.tile([128, D], BF16)
        nc.gpsimd.dma_start(out=a2T_sb, in_=a2T_ext.ap())

        # q/k proj biases: [768] -> [128, 6] each
        qb_sb = consts.tile([128, DT], F32)
        nc.gpsimd.dma_start(
            out=qb_sb, in_=qkvb_ext.ap()[:D].rearrange("(jt p) -> p jt", p=128)
        )
        kb_sb = consts.tile([128, DT], F32)
        nc.gpsimd.dma_start(
            out=kb_sb, in_=qkvb_ext.ap()[D:2 * D].rearrange("(jt p) -> p jt", p=128)
        )
        # v bias broadcast across partitions: [768] -> [128, 768] (bf16)
        vb_src = qkvb_ext.ap()[2 * D:]
        vb_bcast = bass.AP(tensor=vb_src.tensor, offset=vb_src.offset,
                           ap=[[0, 128]] + list(vb_src.ap))
        vbias_sb = consts.tile([128, D], BF16)
        nc.gpsimd.dma_start(out=vbias_sb, in_=vb_bcast)

        a1b_sb = consts.tile([64, 1], F32)
        nc.gpsimd.dma_start(out=a1b_sb, in_=a1b_ext.ap()[:, None])

        eps_sb = consts.tile([128, 1], F32)
        nc.vector.memset(eps_sb, EPS)

        # q-projection weights live through the whole kernel (q deferred)
        qwT_sb = consts.tile([128, DT, D], BF16, tag="qwT")
        nc.gpsimd.dma_start(
            out=qwT_sb, in_=qT_q_ext.ap().rearrange("(ko p) j -> p ko j", p=128)
        )

        # ---------------- big activation tensors ----------------
        qT_sb = big.tile([128, DT, S], BF16, tag="qT")
        kT_sb = big.tile([128, DT, S], BF16, tag="kT")
        v_sb = big.tile([128, ST, H * HB], BF16, tag="v")
        xT_sb = big.tile([128, DT, S], BF16, tag="xT")

        v_blocks = v_sb.rearrange("p t (h u) -> p t h u", u=HB)
        nc.vector.memset(v_blocks[:, :, :, 0:1], 1.0)
        nc.vector.memset(v_blocks[:, :, :, HB - 1:HB], 1.0)

        # ---------------- phase 1: LN + transpose + k/v (+q chunk 0) -------
        with tc.tile_pool(name="p1", bufs=2) as temps, \
             tc.tile_pool(name="p1small", bufs=4) as small, \
             tc.tile_pool(name="kvw", bufs=1) as kvw_pool, \
             tc.tile_pool(name="p1tr", bufs=2, space="PSUM") as psum_tr, \
             tc.tile_pool(name="p1mm", bufs=4, space="PSUM") as psum_p1:

            kvwT_sb = kvw_pool.tile([128, DT, 2 * D], BF16, tag="kvwT")
            nc.sync.dma_start(
                out=kvwT_sb, in_=qT_kv_ext.ap().rearrange("(ko p) j -> p ko j", p=128)
            )

            def q_proj(c, mm_pool):
                """q projection for chunk c: qT[:, :, c*512:(c+1)*512]."""
                cs = c * NC_CHUNK
                for jt in range(DT):
                    qp = mm_pool.tile([128, 512], F32, tag="mm")
                    for kd in range(DT):
                        nc.tensor.matmul(
                            qp,
                            lhsT=qwT_sb[:, kd, jt * 128:(jt + 1) * 128],
                            rhs=xT_sb[:, kd, cs:cs + 512],
                            start=(kd == 0), stop=(kd == DT - 1),
                        )
                    nc.vector.tensor_scalar_add(
                        out=qT_sb[:, jt, cs:cs + 512],
                        in0=qp, scalar1=qb_sb[:, jt:jt + 1],
                    )

            for sc in range(4):
                for st in range(4 * sc, 4 * sc + 4):
                    y_t = temps.tile([128, D], F32, tag="y")
                    nc.sync.dma_start(
                        out=y_t, in_=y_ext[st * 128:(st + 1) * 128, :])

                    stats = small.tile([128, 3, 6], F32, tag="stats")
                    y_grp = y_t.rearrange("p (g c) -> p g c", g=3)
                    for g in range(3):
                        nc.vector.bn_stats(out=stats[:, g, :], in_=y_grp[:, g, :])
                    mv = small.tile([128, 2], F32, tag="mv")
                    nc.vector.bn_aggr(out=mv, in_=stats)

                    rstd = small.tile([128, 1], F32, tag="rstd")
                    nc.scalar.activation(
                        out=rstd, in_=mv[:, 1:2],
                        func=mybir.ActivationFunctionType.Sqrt, bias=eps_sb,
                        scale=1.0,
                    )
                    nc.vector.reciprocal(out=rstd, in_=rstd)
                    x_bf = temps.tile([128, D], BF16, tag="xbf")
                    nc.vector.tensor_scalar(
                        out=x_bf, in0=y_t, scalar1=mv[:, 0:1], scalar2=rstd,
                        op0=mybir.AluOpType.subtract, op1=mybir.AluOpType.mult,
                    )
                    for dt in range(DT):
                        tr = psum_tr.tile([128, 128], BF16, tag="tr")
                        nc.tensor.transpose(
                            tr, x_bf[:, dt * 128:(dt + 1) * 128], ident)
                        nc.scalar.copy(
                            out=xT_sb[:, dt, st * 128:(st + 1) * 128], in_=tr)

                    # v projection for this s-tile (+bias), into 66-blocks
                    for jc, (j0, jw) in enumerate(((0, 512), (512, 256))):
                        vp = psum_p1.tile([128, 512], F32, tag="mm")
                        for kd in range(DT):
                            nc.tensor.matmul(
                                vp[:, :jw],
                                lhsT=xT_sb[:, kd, st * 128:(st + 1) * 128],
                                rhs=kvwT_sb[:, kd, D + j0: D + j0 + jw],
                                start=(kd == 0), stop=(kd == DT - 1),
                            )
                        h0 = j0 // 64
                        nh = jw // 64
                        nc.vector.tensor_add(
                            out=v_blocks[:, st, h0:h0 + nh, 1:65],
                            in0=vp[:, :jw].rearrange("p (h e) -> p h e", e=64),
                            in1=vbias_sb[:, j0:j0 + jw].rearrange(
                                "p (h e) -> p h e", e=64),
                        )

                # k projection for this s-chunk
                for jt in range(DT):
                    kp = psum_p1.tile([128, 512], F32, tag="mm")
                    for kd in range(DT):
                        nc.tensor.matmul(
                            kp,
                            lhsT=kvwT_sb[:, kd, jt * 128:(jt + 1) * 128],
                            rhs=xT_sb[:, kd, sc * 512:(sc + 1) * 512],
                            start=(kd == 0), stop=(kd == DT - 1),
                        )
                    nc.vector.tensor_scalar_add(
                        out=kT_sb[:, jt, sc * 512:(sc + 1) * 512],
                        in0=kp, scalar1=kb_sb[:, jt:jt + 1],
                    )

            q_proj(0, psum_p1)

        # ---------------- attention + fused msa/adapter/output -------------
        with tc.tile_pool(name="sc", bufs=2, space="PSUM") as psum_sc, \
             tc.tile_pool(name="cx", bufs=1, space="PSUM") as psum_cx, \
             tc.tile_pool(name="mm", bufs=2, space="PSUM") as psum_mm, \
             tc.tile_pool(name="et", bufs=5) as e_pool, \
             tc.tile_pool(name="cu", bufs=6) as cu_pool, \
             tc.tile_pool(name="nrm", bufs=3) as nrm_pool, \
             tc.tile_pool(name="nrm2", bufs=3) as nrm2_pool, \
             tc.tile_pool(name="ctxp", bufs=2) as ctx_pool, \
             tc.tile_pool(name="msap", bufs=1) as msa_pool, \
             tc.tile_pool(name="outp", bufs=2) as out_pool, \
             tc.tile_pool(name="dram", bufs=3, space="DRAM") as dram_pool:

            # per-chunk state carried between chunk iterations
            prev = {}

            def emit_attention_hp(c, hp, ctxT):
                cs = c * NC_CHUNK
                pA = psum_cx.tile([65, 512], F32, tag="pA")
                pB = psum_cx.tile([65, 512], F32, tag="pB")
                for t in range(ST):
                    s_t = psum_sc.tile([128, 1024], F32, tag="s")
                    nc.tensor.matmul(
                        s_t[:, 0:512],
                        lhsT=kT_sb[0:64, hp, t * 128:(t + 1) * 128],
                        rhs=qT_sb[0:64, hp, cs:cs + 512],
                        start=True, stop=True, tile_position=(0, 0),
                    )
                    nc.tensor.matmul(
                        s_t[:, 512:1024],
                        lhsT=kT_sb[64:128, hp, t * 128:(t + 1) * 128],
                        rhs=qT_sb[64:128, hp, cs:cs + 512],
                        start=True, stop=True, tile_position=(64, 0),
                    )
                    e_t = e_pool.tile([128, 1024], BF16, tag="et")
                    nc.scalar.activation(
                        out=e_t, in_=s_t, func=EXP,
                        scale=float(1.0 / np.sqrt(DH)),
                    )
                    vblk = v_sb[:, t, :]
                    hA, hB = 2 * hp, 2 * hp + 1
                    nc.tensor.matmul(
                        pA,
                        lhsT=vblk[:, hA * HB + 1: hA * HB + HB],
                        rhs=e_t[:, 0:512],
                        start=(t == 0), stop=(t == ST - 1),
                    )
                    nc.tensor.matmul(
                        pB,
                        lhsT=vblk[:, hB * HB + 1: hB * HB + HB],
                        rhs=e_t[:, 512:1024],
                        start=(t == 0), stop=(t == ST - 1),
                    )
                # eager evacuation (frees pA/pB for next hp); row 64 = denom
                cuA = cu_pool.tile([65, 512], BF16, tag="cu")
                nc.vector.tensor_copy(out=cuA, in_=pA)
                cuB = cu_pool.tile([65, 512], BF16, tag="cu")
                nc.vector.tensor_copy(out=cuB, in_=pB)
                # per-hp normalize: reciprocal of the two denominator rows,
                # broadcast across 64 partitions via DRAM round-trip DMA,
                # then elementwise multiply into ctxT (B half via partition-
                # shift DMA staging).
                rinvA = nrm2_pool.tile([1, 512], BF16, tag="rinvA")
                rinvB = nrm2_pool.tile([1, 512], BF16, tag="rinvB")
                with nc.allow_low_precision(reason="softmax denom recip bf16"):
                    nc.vector.reciprocal(out=rinvA, in_=cuA[64:65, :])
                    nc.vector.reciprocal(out=rinvB, in_=cuB[64:65, :])
                rd = dram_pool.tile([2, 512], BF16, tag="rdram")
                nc.gpsimd.dma_start(out=rd[0:1, :], in_=rinvA)
                nc.gpsimd.dma_start(out=rd[1:2, :], in_=rinvB)
                rbc = nrm_pool.tile([64, 2, 512], BF16, tag="rbc")
                bc_src = bass.AP(tensor=rd.tensor, offset=rd.offset,
                                 ap=[[0, 64]] + list(rd.ap))
                nc.gpsimd.dma_start(out=rbc, in_=bc_src)
                nc.vector.tensor_mul(
                    out=ctxT[0:64, hp, :], in0=cuA[0:64, :], in1=rbc[:, 0, :])
                stgB = nrm2_pool.tile([64, 512], BF16, tag="stgB")
                nc.vector.tensor_mul(
                    out=stgB, in0=cuB[0:64, :], in1=rbc[:, 1, :])
                nc.gpsimd.dma_start(out=ctxT[64:128, hp, :], in_=stgB)

            def filler_msa_et(c, et_i, ctxT, msaT):
                cs = c * NC_CHUNK
                mp = psum_mm.tile([128, 512], F32, tag="mm")
                for kd in range(DT):
                    nc.tensor.matmul(
                        mp,
                        lhsT=msaT_sb_w[:, kd, et_i * 128:(et_i + 1) * 128],
                        rhs=ctxT[:, kd, :],
                        start=(kd == 0), stop=(kd == DT - 1),
                    )
                nc.vector.tensor_copy(out=msaT[:, et_i, :], in_=mp)

            def filler_adapter(c, msaT, hT):
                hp_ps = psum_mm.tile([128, 512], F32, tag="mm")
                for kd in range(DT):
                    nc.tensor.matmul(
                        hp_ps[0:64, :],
                        lhsT=a1T_sb[:, kd, :],
                        rhs=msaT[:, kd, :],
                        start=(kd == 0), stop=(kd == DT - 1),
                    )
                nc.scalar.activation(
                    out=hT[0:64, :], in_=hp_ps[0:64, :], func=RELU,
                    bias=a1b_sb, scale=1.0,
                )

            def filler_out_st(c, sti, msaT, hT):
                """Output assembly for s-tile (4c+sti): transpose msaT back,
                adapter second matmul, residual add, DMA out."""
                st = 4 * c + sti
                msan = out_pool.tile([128, D], F32, tag="msan")
                for dt in range(DT):
                    tr = psum_mm.tile([128, 128], BF16, tag="mm")
                    nc.tensor.transpose(
                        tr, msaT[:, dt, sti * 128:(sti + 1) * 128], ident)
                    nc.vector.tensor_copy(
                        out=msan[:, dt * 128:(dt + 1) * 128], in_=tr)
                o_t = out_pool.tile([128, D], F32, tag="out")
                ad1 = psum_mm.tile([128, 512], F32, tag="mm")
                nc.tensor.matmul(
                    ad1,
                    lhsT=hT[:, sti * 128:(sti + 1) * 128],
                    rhs=a2T_sb[:, 0:512],
                    start=True, stop=True,
                )
                nc.vector.tensor_add(out=o_t[:, 0:512], in0=ad1, in1=msan[:, 0:512])
                ad2 = psum_mm.tile([128, 512], F32, tag="mm")
                nc.tensor.matmul(
                    ad2[:, 0:256],
                    lhsT=hT[:, sti * 128:(sti + 1) * 128],
                    rhs=a2T_sb[:, 512:768],
                    start=True, stop=True,
                )
                nc.vector.tensor_add(
                    out=o_t[:, 512:768], in0=ad2[:, 0:256], in1=msan[:, 512:768])
                nc.sync.dma_start(out=out_ext[st * 128:(st + 1) * 128, :], in_=o_t)

            def emit_output_phase_slices(c_prev, ctxT, msaT, hT):
                """Build the list of 6 filler closures for chunk c_prev's
                msa/adapter/output, to interleave into the next chunk's hps."""
                # NOTE: emission order is program order — the adapter (which
                # writes hT) MUST be emitted before any out-st work reads hT.
                slices = [[] for _ in range(6)]
                slices[0].append(lambda: filler_msa_et(c_prev, 0, ctxT, msaT))
                slices[1].append(lambda: filler_msa_et(c_prev, 1, ctxT, msaT))
                slices[2].append(lambda: filler_msa_et(c_prev, 2, ctxT, msaT))
                slices[2].append(lambda: filler_msa_et(c_prev, 3, ctxT, msaT))
                slices[3].append(lambda: filler_msa_et(c_prev, 4, ctxT, msaT))
                slices[3].append(lambda: filler_msa_et(c_prev, 5, ctxT, msaT))
                slices[4].append(lambda: filler_adapter(c_prev, msaT, hT))
                slices[4].append(lambda: filler_out_st(c_prev, 0, msaT, hT))
                slices[4].append(lambda: filler_out_st(c_prev, 1, msaT, hT))
                slices[5].append(lambda: filler_out_st(c_prev, 2, msaT, hT))
                slices[5].append(lambda: filler_out_st(c_prev, 3, msaT, hT))
                return slices

            for c in range(NCHUNK):
                ctxT_c = ctx_pool.tile([128, DT, 512], BF16, tag="ctxT")

                # build filler slices from previous chunk
                slices = [[] for _ in range(6)]
                if c > 0:
                    msaT = msa_pool.tile([128, DT, 512], BF16, tag="msaT")
                    hT = msa_pool.tile([128, 512], BF16, tag="hT")
                    nc.vector.memset(hT[64:128, :], 0.0)
                    nc.vector.memset(hT[64:65, :], 1.0)
                    ms = emit_output_phase_slices(c - 1, prev["ctxT"], msaT, hT)
                    for i in range(6):
                        slices[i].extend(ms[i])
                if c < NCHUNK - 1:
                    # q projection for chunk c+1: 2 jt per slice over 3 slices
                    for i in range(3):
                        def qslice(i=i, cq=c + 1):
                            for jt in (2 * i, 2 * i + 1):
                                qp = psum_mm.tile([128, 512], F32, tag="mm")
                                for kd in range(DT):
                                    nc.tensor.matmul(
                                        qp,
                                        lhsT=qwT_sb[:, kd, jt * 128:(jt + 1) * 128],
                                        rhs=xT_sb[:, kd, cq * 512:(cq + 1) * 512],
                                        start=(kd == 0), stop=(kd == DT - 1),
                                    )
                                nc.vector.tensor_scalar_add(
                                    out=qT_sb[:, jt, cq * 512:(cq + 1) * 512],
                                    in0=qp, scalar1=qb_sb[:, jt:jt + 1],
                                )
                        slices[i].append(qslice)

                for hp in range(DT):
                    emit_attention_hp(c, hp, ctxT_c)
                    for fn in slices[hp]:
                        fn()

                prev["ctxT"] = ctxT_c

            # tail: msa/adapter/output for the last chunk
            msaT = msa_pool.tile([128, DT, 512], BF16, tag="msaT")
            hT = msa_pool.tile([128, 512], BF16, tag="hT")
            nc.vector.memset(hT[64:65, :], 1.0)
            for et_i in range(DT):
                filler_msa_et(NCHUNK - 1, et_i, prev["ctxT"], msaT)
            filler_adapter(NCHUNK - 1, msaT, hT)
            for sti in range(4):
                filler_out_st(NCHUNK - 1, sti, msaT, hT)


_NC_CACHE = None


def _get_nc():
    global _NC_CACHE
    if _NC_CACHE is None:
        _NC_CACHE = build_nc()
    return _NC_CACHE


def _prep_in_maps(y, ln_g, ln_b, qkv_w, qkv_b, msa_w, a1_w, a1_b, a2_w, a2_b):
    f = np.float32
    y = np.asarray(y, f)
    ln_g = np.asarray(ln_g, f)
    ln_b = np.asarray(ln_b, f)
    qkv_w = np.asarray(qkv_w, f)
    qkv_b = np.asarray(qkv_b, f)
    msa_w = np.asarray(msa_w, f)
    a1_w = np.asarray(a1_w, f)
    a1_b = np.asarray(a1_b, f)
    a2_w = np.asarray(a2_w, f)
    a2_b = np.asarray(a2_b, f)

    import ml_dtypes
    bf = ml_dtypes.bfloat16

    # Fold LN affine into QKV: (g*xn + b) @ W.T + c == xn @ (W*g).T + (W@b + c)
    qkv_wT = np.ascontiguousarray((qkv_w * ln_g[None, :]).T)          # [768, 2304]
    qkv_b_eff = (qkv_b + qkv_w @ ln_b).astype(f)                      # [2304]
    qkv_wT_q = np.ascontiguousarray(qkv_wT[:, :D]).astype(bf)
    qkv_wT_kv = np.ascontiguousarray(qkv_wT[:, D:]).astype(bf)
    msa_wT = np.ascontiguousarray(msa_w.T).astype(bf)                 # [768, 768]
    a1_wT = np.ascontiguousarray(a1_w.T).astype(bf)                   # [768, 64]
    a2_aug = np.zeros((128, D), f)                                    # [128, 768]
    a2_aug[:R] = a2_w.T
    a2_aug[R] = a2_b
    a2_aug = a2_aug.astype(bf)

    shared = {
        "qkv_wT_q": qkv_wT_q, "qkv_wT_kv": qkv_wT_kv, "qkv_b_eff": qkv_b_eff,
        "msa_wT": msa_wT, "a1_wT": a1_wT, "a1_b": a1_b, "a2_wT_aug": a2_aug,
    }
    in_maps = [dict(shared, y=np.ascontiguousarray(y[b])) for b in range(NCORES)]
    return in_maps


def run(trace=False, **inputs):
    in_maps = _prep_in_maps(**inputs)
    nc = _get_nc()
    res = bass_utils.run_bass_kernel_spmd(
        nc, in_maps, core_ids=list(range(NCORES)), trace=trace
    )
    out = np.stack([r["out"] for r in res.results], axis=0)
    return out.astype(np.float32), res


def kernel(**inputs) -> np.ndarray:
    out, _ = run(trace=False, **inputs)
    return out
